# revision 19
# baseline (speedup 1.0000x reference)
"""Trainium2 Bass kernel for nn_AdaptiveNodeClassifier — V2 architecture.

Nodes sharded across 8 cores (dest-owner edge partition). Per layer:
z' = dinv*(feat@W) per shard -> AllGather -> bf16 table [100352,128] in DRAM.
Edges are arranged dest-tile-major with a globally uniform chunk count m per
dest tile (static SPMD schedule); per 128-edge chunk, source rows are fetched
with indirect_dma_start (standard DGE dynamic DMA) and scatter-added into the
dest tile's PSUM accumulator via a one-hot (dst%128) matmul on TensorE.
Self-loop term is added from the local z' tile. Head: low/high mix, 2-layer
MLP, indexed injection, log_softmax — all per dest tile on-chip.
"""

from dataclasses import dataclass, field

import numpy as np
import ml_dtypes

import concourse.bacc as bacc
import concourse.bass as bass
import concourse.mybir as mybir
import concourse.tile as tile

F32 = mybir.dt.float32
BF16 = mybir.dt.bfloat16
I32 = mybir.dt.int32

HID = 128
C = 10
LAM = 0.8


@dataclass
class Cfg:
    ncores: int = 8
    npc: int = 12500
    m: int = 0          # chunks of 128 edges per dest tile (set by host plan)
    gform: str = "B"    # "A": one [128,m] gather per tile; "B": m [128,1] gathers
    perms: list = field(default_factory=list)  # per-core old->new local node id

    @property
    def npc_pad(self):
        return (self.npc + 127) // 128 * 128 if self.npc % 128 else self.npc + 128

    @property
    def nt(self):
        return self.npc_pad // 128

    @property
    def n(self):
        return self.ncores * self.npc


def build_host_plan(cfg: Cfg, inputs: dict) -> list[dict]:
    x = np.asarray(inputs["x"], np.float32)
    ei = np.asarray(inputs["edge_index"]).astype(np.int64)
    inj = np.asarray(inputs["inject_indices"]).astype(np.int64)
    preds = np.asarray(inputs["initial_preds"], np.float32)
    n, npc, npp = cfg.n, cfg.npc, cfg.npc_pad

    src, dst = ei[0], ei[1]
    deg = np.bincount(dst, minlength=n).astype(np.float32) + 1.0
    dinv = (1.0 / np.sqrt(deg)).astype(np.float32)

    owner = dst // npc
    # Rebalance nodes across dest tiles per core (LPT, <=128 nodes/tile)
    # to minimize the max per-tile edge count -> smaller m.
    dstl = dst % npc
    cfg.perms = []       # per core: old local id -> new local id (in [0, npc_pad))
    maxcnt = 0
    for c in range(cfg.ncores):
        degl = np.bincount(dstl[owner == c], minlength=npc)
        order = np.argsort(-degl, kind="stable")
        bin_load = np.zeros(cfg.nt, np.int64)
        bin_n = np.zeros(cfg.nt, np.int64)
        newid = np.zeros(npc, np.int64)
        for i in order:
            open_b = bin_n < 128
            b = np.flatnonzero(open_b)[np.argmin(bin_load[open_b])]
            newid[i] = b * 128 + bin_n[b]
            bin_n[b] += 1
            bin_load[b] += degl[i]
        cfg.perms.append(newid)
        maxcnt = max(maxcnt, int(bin_load.max()))
    cfg.m = (maxcnt + 127) // 128

    mask = np.zeros(n, np.float32)
    mask[np.unique(inj)] = 1.0

    # global source row in the AllGathered (permuted) table
    perm_all = np.concatenate(
        [c * npp + cfg.perms[c] for c in range(cfg.ncores)])
    src_row = perm_all[src]

    in_maps = []
    for c in range(cfg.ncores):
        sl = slice(c * npc, (c + 1) * npc)
        perm = cfg.perms[c]                      # old -> new (0..npp)
        perm_inv = np.full(npp, npc, np.int64)   # new -> old (pad -> npc)
        perm_inv[perm] = np.arange(npc)
        perm_inv = np.minimum(perm_inv, npc - 1)
        occupied = np.zeros(npp, bool)
        occupied[perm] = True
        xt = np.zeros((128, npp), np.float32)
        xt[:, occupied] = x[sl].T[:, perm_inv[occupied]]
        tmp = np.ones(npp, np.float32)
        tmp[occupied] = dinv[sl][perm_inv[occupied]]
        dv = np.ascontiguousarray(tmp.reshape(cfg.nt, 128).T)
        tmp = np.zeros(npp, np.float32)
        tmp[occupied] = mask[sl][perm_inv[occupied]]
        mk = np.ascontiguousarray(tmp.reshape(cfg.nt, 128).T)
        pr = np.zeros((npp, C), np.float32)
        pr[occupied] = preds[sl][perm_inv[occupied]]
        pt = np.ascontiguousarray(
            pr.reshape(cfg.nt, 128, C).transpose(1, 0, 2)).reshape(128, cfg.nt * C)

        # edge schedule: dest-tile-major, m*128 slots per tile
        oc = owner == c
        dnew = perm[dstl[oc]]                 # new local id of each edge dst
        rows_all = src_row[oc]
        dt_new = dnew // 128
        lo_new = (dnew % 128).astype(np.float32)
        gidx = np.zeros((128, cfg.nt * cfg.m), np.int64)
        dlo = np.full((128, cfg.nt * cfg.m), -1.0, np.float32)
        for d in range(cfg.nt):
            mmask = dt_new == d
            k = int(mmask.sum())
            pp = np.arange(k) % 128
            jj = np.arange(k) // 128
            gidx[pp, d * cfg.m + jj] = rows_all[mmask]
            dlo[pp, d * cfg.m + jj] = lo_new[mmask]
        im = {
            "xt": xt.astype(ml_dtypes.bfloat16),
            "dinv": dv,
            "maskt": mk,
            "predst": pt,
            "gidx": gidx.astype(np.int32),
            "dstlo": dlo.astype(np.float32),
            "iota": np.broadcast_to(
                np.arange(128, dtype=np.float32),
                (128, 128)).astype(ml_dtypes.bfloat16).copy(),
            "identb": np.eye(128, dtype=ml_dtypes.bfloat16),
            "identf": np.eye(128, dtype=np.float32),
            "w1": np.asarray(inputs["W1"], np.float32),
            "w2": np.asarray(inputs["W2"], np.float32),
            "wl1": np.asarray(inputs["Wl1"], np.float32),
            "wl2": np.asarray(inputs["Wl2"], np.float32),
            "wm1": np.asarray(inputs["Wm1"], np.float32),
            "wm2": np.asarray(inputs["Wm2"], np.float32),
            "b1r": np.tile(np.asarray(inputs["b1"], np.float32), (128, 1)),
            "b2r": np.tile(np.asarray(inputs["b2"], np.float32), (128, 1)),
            "bm1c": np.asarray(inputs["bm1"], np.float32).reshape(128, 1),
            "bm2c": np.asarray(inputs["bm2"], np.float32).reshape(C, 1),
        }
        in_maps.append(im)
    return in_maps


def build_graph(cfg: Cfg) -> bacc.Bacc:
    nc = bacc.Bacc("TRN2", target_bir_lowering=False, debug=False,
                   num_devices=cfg.ncores)
    npp, nt, m = cfg.npc_pad, cfg.nt, cfg.m

    xt_d = nc.dram_tensor("xt", [128, npp], BF16, kind="ExternalInput")
    dinv_d = nc.dram_tensor("dinv", [128, nt], F32, kind="ExternalInput")
    mask_d = nc.dram_tensor("maskt", [128, nt], F32, kind="ExternalInput")
    preds_d = nc.dram_tensor("predst", [128, nt * C], F32, kind="ExternalInput")
    gidx_d = nc.dram_tensor("gidx", [128, nt * m], I32, kind="ExternalInput")
    dstlo_d = nc.dram_tensor("dstlo", [128, nt * m], F32, kind="ExternalInput")
    iota_d = nc.dram_tensor("iota", [128, 128], BF16, kind="ExternalInput")
    identb_d = nc.dram_tensor("identb", [128, 128], BF16, kind="ExternalInput")
    identf_d = nc.dram_tensor("identf", [128, 128], F32, kind="ExternalInput")
    wd = {k: nc.dram_tensor(k, [128, 128], F32, kind="ExternalInput")
          for k in ["w1", "w2", "wl1", "wl2", "wm1"]}
    wm2_d = nc.dram_tensor("wm2", [128, C], F32, kind="ExternalInput")
    b1r_d = nc.dram_tensor("b1r", [128, 128], F32, kind="ExternalInput")
    b2r_d = nc.dram_tensor("b2r", [128, 128], F32, kind="ExternalInput")
    bm1c_d = nc.dram_tensor("bm1c", [128, 1], F32, kind="ExternalInput")
    bm2c_d = nc.dram_tensor("bm2c", [C, 1], F32, kind="ExternalInput")
    out_d = nc.dram_tensor("out", [npp, C], F32, kind="ExternalOutput")

    zloc = [nc.dram_tensor(f"z{i}loc", [npp, HID], BF16) for i in (1, 2)]
    ztab = [nc.dram_tensor(f"ztab{i}", [cfg.ncores * npp, HID], BF16,
                           addr_space="Shared") for i in (1, 2)]
    rg = [list(range(cfg.ncores))]

    with tile.TileContext(nc) as tc:
        with (
            tc.tile_pool(name="const", bufs=1) as const,
            tc.tile_pool(name="work", bufs=4) as work,
            tc.tile_pool(name="msg", bufs=3) as msgp,
            tc.tile_pool(name="psum", bufs=2, space="PSUM") as psum,
            tc.tile_pool(name="psumb", bufs=2, space="PSUM") as psumb,
        ):
            def load_const(dram, shape, dtype=F32):
                t = const.tile(shape, dtype, tag=dram.name, name=f"{dram.name}_sb")
                nc.sync.dma_start(t[:], dram[:])
                return t

            dinv_t = load_const(dinv_d, [128, nt])
            mask_t = load_const(mask_d, [128, nt])
            preds_t = load_const(preds_d, [128, nt * C])
            gidx_t = load_const(gidx_d, [128, nt * m], I32)
            dstlo_t = load_const(dstlo_d, [128, nt * m])
            iota_t = load_const(iota_d, [128, 128], BF16)
            identb = load_const(identb_d, [128, 128], BF16)
            identf = load_const(identf_d, [128, 128], F32)
            b1r_t = load_const(b1r_d, [128, 128])
            b2r_t = load_const(b2r_d, [128, 128])
            bm1c_t = load_const(bm1c_d, [128, 1])
            bm2c_t = load_const(bm2c_d, [C, 1])

            wb = {}
            for k in ["w1", "w2", "wl1", "wl2"]:
                wf = work.tile([128, 128], F32, tag="wload", name="wf")
                nc.sync.dma_start(wf[:], wd[k][:])
                wb[k] = const.tile([128, 128], BF16, tag=f"{k}b", name=f"{k}b")
                nc.vector.tensor_copy(wb[k][:], wf[:])
            wf = work.tile([128, 128], F32, tag="wload", name="wf1")
            nc.sync.dma_start(wf[:], wd["wm1"][:])
            wm1b = const.tile([128, 128], BF16, tag="wm1b", name="wm1b")
            nc.scalar.activation(wm1b[:], wf[:],
                                 mybir.ActivationFunctionType.Copy, scale=0.5)
            wf = work.tile([128, C], F32, tag="wm2load", name="wf2")
            nc.sync.dma_start(wf[:], wm2_d[:])
            wm2b = const.tile([128, C], BF16, tag="wm2b", name="wm2b")
            nc.vector.tensor_copy(wm2b[:], wf[:])

            xtb = const.tile([128, npp], BF16, tag="xtb", name="xtb")
            nc.sync.dma_start(xtb[:], xt_d[:])

            htb = const.tile([128, npp], BF16, tag="htb", name="htb")

            def make_table(lhsT, w_t, z_d):
                for d in range(nt):
                    ps = psum.tile([128, 128], F32, tag="ps", name="ps")
                    nc.tensor.matmul(ps[:], lhsT[:, d * 128:(d + 1) * 128], w_t[:])
                    zb = work.tile([128, 128], BF16, tag="ztile", name="zb")
                    nc.vector.tensor_scalar(
                        zb[:], ps[:], dinv_t[:, d:d + 1], None,
                        mybir.AluOpType.mult)
                    nc.sync.dma_start(z_d[d * 128:(d + 1) * 128, :], zb[:])

            def seg_psum(tab, d):
                """Gather + one-hot matmul accumulate dest tile d -> psum tile."""
                ps = psum.tile([128, 128], F32, tag="acc", name="acc")
                if cfg.gform == "A":
                    mt = msgp.tile([128, m, 128], BF16, tag="msg", name="mt")
                    nc.gpsimd.indirect_dma_start(
                        out=mt[:], out_offset=None, in_=tab[:],
                        in_offset=bass.IndirectOffsetOnAxis(
                            ap=gidx_t[:, d * m:(d + 1) * m], axis=0))
                for j in range(m):
                    col = d * m + j
                    if cfg.gform == "B":
                        mtj = msgp.tile([128, 128], BF16, tag="msg", name="mtj",
                                        bufs=12)
                        nc.gpsimd.indirect_dma_start(
                            out=mtj[:], out_offset=None, in_=tab[:],
                            in_offset=bass.IndirectOffsetOnAxis(
                                ap=gidx_t[:, col:col + 1], axis=0))
                        rhs = mtj[:]
                    else:
                        rhs = mt[:, j, :]
                    oh = work.tile([128, 128], BF16, tag="oh", name="oh", bufs=8)
                    nc.vector.tensor_scalar(
                        oh[:], iota_t[:], dstlo_t[:, col:col + 1], None,
                        mybir.AluOpType.is_equal)
                    nc.tensor.matmul(ps[:], oh[:], rhs,
                                     start=(j == 0), stop=(j == m - 1))
                return ps

            def layer(tab, z_d, lhsT_next, w_high, b_t, relu):
                """Per dest tile: low = dinv*(seg+zl)+b; out = f(0.5*(low+high))."""
                for d in range(nt):
                    dsl = slice(d * 128, (d + 1) * 128)
                    ps = seg_psum(tab, d)
                    zl = work.tile([128, 128], BF16, tag="zl", name="zl")
                    nc.sync.dma_start(zl[:], z_d[dsl, :])
                    zlf = work.tile([128, 128], F32, tag="zlf", name="zlf")
                    nc.vector.tensor_copy(zlf[:], zl[:])
                    v = work.tile([128, 128], F32, tag="v", name="v")
                    nc.vector.tensor_tensor(v[:], ps[:], zlf[:],
                                            mybir.AluOpType.add)
                    low = work.tile([128, 128], F32, tag="low", name="low")
                    nc.vector.tensor_scalar(low[:], v[:], dinv_t[:, d:d + 1],
                                            None, mybir.AluOpType.mult)
                    u = work.tile([128, 128], F32, tag="u", name="u")
                    nc.vector.tensor_tensor(u[:], low[:], b_t[:],
                                            mybir.AluOpType.add)
                    hp = psum.tile([128, 128], F32, tag="ps", name="hp")
                    nc.tensor.matmul(hp[:], lhsT_next[:, dsl], w_high[:])
                    w = work.tile([128, 128], F32, tag="w", name="w")
                    nc.vector.tensor_tensor(w[:], u[:], hp[:],
                                            mybir.AluOpType.add)
                    yield d, dsl, w

            # ---------- layer 1 ----------
            make_table(xtb, wb["w1"], zloc[0])
            nc.gpsimd.collective_compute(
                "AllGather", mybir.AluOpType.bypass,
                ins=[zloc[0][:]], outs=[ztab[0][:]], replica_groups=rg)
            for d, dsl, w in layer(ztab[0], zloc[0], xtb, wb["wl1"], b1r_t, True):
                hb = work.tile([128, 128], BF16, tag="hb", name="hb")
                nc.scalar.activation(hb[:], w[:],
                                     mybir.ActivationFunctionType.Relu, scale=0.5)
                pt = psumb.tile([128, 128], BF16, tag="ptb", name="ptb")
                nc.tensor.transpose(pt[:], hb[:], identb[:])
                nc.vector.tensor_copy(htb[:, dsl], pt[:])

            # ---------- layer 2 ----------
            make_table(htb, wb["w2"], zloc[1])
            nc.gpsimd.collective_compute(
                "AllGather", mybir.AluOpType.bypass,
                ins=[zloc[1][:]], outs=[ztab[1][:]], replica_groups=rg)
            for d, dsl, w in layer(ztab[1], zloc[1], htb, wb["wl2"], b2r_t, False):
                h2p = work.tile([128, 128], BF16, tag="h2p", name="h2p")
                nc.scalar.activation(h2p[:], w[:],
                                     mybir.ActivationFunctionType.Copy)
                pt = psumb.tile([128, 128], BF16, tag="ptb", name="ptb2")
                nc.tensor.transpose(pt[:], h2p[:], identb[:])
                h2pt = work.tile([128, 128], BF16, tag="h2pt", name="h2pt")
                nc.vector.tensor_copy(h2pt[:], pt[:])
                t1p = psum.tile([128, 128], F32, tag="hd", name="t1p", bufs=2)
                nc.tensor.matmul(t1p[:], wm1b[:], h2pt[:])
                t1t = work.tile([128, 128], BF16, tag="t1t", name="t1t")
                nc.scalar.activation(t1t[:], t1p[:],
                                     mybir.ActivationFunctionType.Relu,
                                     bias=bm1c_t[:])
                lgp = psum.tile([C, 128], F32, tag="hd", name="lgp", bufs=2)
                nc.tensor.matmul(lgp[:], wm2b[:], t1t[:])
                lgt = work.tile([C, 128], F32, tag="lgt", name="lgt")
                nc.vector.tensor_scalar(lgt[:], lgp[:], bm2c_t[:], None,
                                        mybir.AluOpType.add)
                ptl = psum.tile([128, C], F32, tag="hd", name="ptl", bufs=2)
                nc.tensor.transpose(ptl[:], lgt[:], identf[:C, :C])
                inj = work.tile([128, C], F32, tag="inj", name="inj")
                nc.vector.tensor_scalar(
                    inj[:], preds_t[:, d * C:(d + 1) * C], mask_t[:, d:d + 1],
                    LAM, mybir.AluOpType.mult, mybir.AluOpType.mult)
                lg = work.tile([128, C], F32, tag="lg", name="lg")
                nc.vector.tensor_tensor(lg[:], ptl[:], inj[:],
                                        mybir.AluOpType.add)
                mneg = work.tile([128, 1], F32, tag="mneg", name="mneg")
                nc.vector.tensor_reduce(mneg[:], lg[:], mybir.AxisListType.X,
                                        mybir.AluOpType.max, negate=True)
                e = work.tile([128, C], F32, tag="e", name="e")
                nc.scalar.activation(e[:], lg[:],
                                     mybir.ActivationFunctionType.Exp,
                                     bias=mneg[:])
                s = work.tile([128, 1], F32, tag="s", name="s")
                nc.vector.tensor_reduce(s[:], e[:], mybir.AxisListType.X,
                                        mybir.AluOpType.add)
                ls = work.tile([128, 1], F32, tag="ls", name="ls")
                nc.scalar.activation(ls[:], s[:],
                                     mybir.ActivationFunctionType.Ln)
                o = work.tile([128, C], F32, tag="o", name="o")
                nc.vector.tensor_scalar(o[:], lg[:], mneg[:], ls[:],
                                        mybir.AluOpType.add,
                                        mybir.AluOpType.subtract)
                nc.sync.dma_start(out_d[d * 128:(d + 1) * 128, :], o[:])

    nc.compile()
    return nc


def kernel(**inputs) -> np.ndarray:
    from concourse.bass_utils import run_bass_kernel_spmd

    cfg = Cfg()
    in_maps = build_host_plan(cfg, inputs)
    nc = build_graph(cfg)
    res = run_bass_kernel_spmd(nc, in_maps, core_ids=list(range(cfg.ncores)))
    return assemble(cfg, [res.results[c]["out"] for c in range(cfg.ncores)])


def assemble(cfg, outs) -> np.ndarray:
    return np.concatenate(
        [outs[c][cfg.perms[c]] for c in range(cfg.ncores)], 0)
